# revision 4
# baseline (speedup 1.0000x reference)
"""Trainium2 Bass kernel v2 for nn_AttentionSpatial (spatial cosine attention).

Per head h (8 heads, head h -> core h):
  q = W_q X, k/v = W_kv Y                (channel-major, [8, 4096])
  q' = q/|q| * temp, k' = k/|k|          (norms via ones-matmul + ACT ln/exp)
  S[key, qcol] = k'.T q'                 (row-packed: 4 key-blocks concurrent
                                          in PE row groups 0/32/64/96)
  P = exp(S)                             (bounded logits => no max pass)
  O = [V | 1].T P                        (fp8 DoubleRow pairs two key blocks
                                          per matmul, or f32r classic)
  out = (W_out O) / den                  (den broadcast via K=1 matmul)

Layouts:
  Qst/Kst [104, 4096] f32r: channels replicated at partition 0/32/64/96
    (produced directly by stacked projection weights wq4/wk4 [64, 104]).
  Vaug fp8 [128, 16, 2, 16]: token-major value pairs (+ones col 8) for
    DoubleRow; or f32r [128, 32, 9] for classic mode.
"""

import numpy as np

import concourse.bass as bass
import concourse.tile as tile
from concourse import mybir

NUM_HEADS = 8
DIM = 64
HD = 8
N = 4096
NCHUNK = 8  # 512-column chunks
QC = 512
NQC = N // QC
NKB = 32  # 128-key blocks
F32 = mybir.dt.float32
F32R = mybir.dt.float32r
FP8 = mybir.dt.float8e4
U8 = mybir.dt.uint8

import os

O_FP8 = os.environ.get("KERN_O_FP8", "1") == "1"

_patched = False


def _apply_walrus_compat():
    """This container's walrus build rejects Drain instructions that carry
    sync waits ("Too many sync wait commands").  Replace multi-engine
    barriers with the sem-only variant and re-emit the TileContext tail
    drain's waits as standalone EventSemaphore instructions."""
    global _patched
    if _patched:
        return
    _patched = True
    from concourse.vector_clock import ScopedClock

    def meb(self, engines):
        for e in engines:
            self.engines[e].drain()
        for inst in self._sem_only_all_engine_barrier_insts("meb"):
            self.engines[inst.engine].add_instruction(inst)

    bass.Bass.multi_engine_barrier = meb

    def _drain_and_barrier(self, tick_clock, wait_clock):
        nc = self.nc
        carrier = nc.sync.nop()
        wait_clock.add_sem_waits(
            carrier.ins, ScopedClock({None: tick_clock.global_clock})
        )
        si = carrier.ins.sync_info
        waits = list(si.on_wait) if si and si.on_wait else []
        if si is not None:
            si.on_wait = []
        sems = list(self.sems.allocated().values())
        placeholder = sems[0] if sems else nc.alloc_semaphore("tailw")
        for w in waits:
            assert w.wait_mode in ("sem-ge-imm", "sem-ge"), w.wait_mode
            ev = nc.sync.wait_ge(placeholder, 0)
            ev.ins.sync_info.on_wait = [w]
        nc.sync.drain()
        nc.all_engine_barrier()
        popped = nc._tile_sem_poison_stack.pop()
        assert popped is self._sem_poison
        nc.clear_and_free_semaphores(list(self.sems.allocated().values()))
        nc.all_engine_barrier()

    tile.TileContext._drain_and_barrier = _drain_and_barrier

    orig_commit = tile.TileContext._commit_instruction

    def _commit_instruction(self, inst, lazy_reg_writes=True):
        si = inst.sync_info
        if si is not None and si.on_wait:
            is_drain = type(inst).__name__ == "InstDrain"
            waits = list(si.on_wait)
            n_ge = sum(
                1 for w in waits if w.wait_mode in ("sem-ge-imm", "sem-ge")
            )
            assert n_ge == len(waits) or not is_drain, f"eq-wait on drain {inst}"
            keep = 0 if is_drain else 1
            if len(waits) > keep and inst.engine != mybir.EngineType.Unassigned:
                kept, split = waits[:keep], waits[keep:]
                si.on_wait = kept
                sems = list(self.sems.allocated().values())
                placeholder = sems[0] if sems else self.nc.alloc_semaphore("splitw")
                eng = self.nc.engines[inst.engine]
                for w in split:
                    assert w.wait_mode in ("sem-ge-imm", "sem-ge"), w.wait_mode
                    ev = eng.wait_ge(placeholder, 0)
                    ev.ins.sync_info.on_wait = [w]
        return orig_commit(self, inst, lazy_reg_writes)

    tile.TileContext._commit_instruction = _commit_instruction


def _emit_rep(nc, tc, rep, x_d, y_d, wqkv_d, wot_d, bc_d, out_d, pools):
    Exp = mybir.ActivationFunctionType.Exp
    Ln = mybir.ActivationFunctionType.Ln
    Copy = mybir.ActivationFunctionType.Copy
    sb, pS, pO, pF, pB, Pp, epi = pools

    # ---- load inputs ----
    X = sb.tile([DIM, N], F32R, tag="X")
    Y = sb.tile([DIM, N], F32R, tag="Y")
    WQKV = sb.tile([DIM, 216], F32R, tag="WQKV")  # wq4 | wk4 | wv
    WOT = sb.tile([HD + 1, DIM + 1], F32R, tag="WOT")
    BC = sb.tile([1, 211], F32R, tag="BC")  # bcq(104) | bck(104) | temp | ones(2)
    nc.sync.dma_start(X[0:32, :].bitcast(F32), x_d[0:32, :])
    nc.gpsimd.dma_start(X[32:64, :].bitcast(F32), x_d[32:64, :])
    nc.sync.dma_start(Y[0:32, :].bitcast(F32), y_d[0:32, :])
    nc.gpsimd.dma_start(Y[32:64, :].bitcast(F32), y_d[32:64, :])
    nc.sync.dma_start(WQKV[:].bitcast(F32), wqkv_d[:])
    nc.sync.dma_start(WOT[:].bitcast(F32), wot_d[:])
    nc.sync.dma_start(BC[:].bitcast(F32), bc_d[:])
    ones8 = sb.tile([HD, 1], F32R, tag="ones8")
    nc.vector.memset(ones8[:].bitcast(F32), 1.0)

    # persistent SBUF state
    Qst = sb.tile([104, N], F32R, tag="Qst")
    Kst = sb.tile([104, N], F32R, tag="Kst")
    Vc = sb.tile([HD, N], F32R, tag="Vc")
    SQq = sb.tile([HD, N], F32, tag="SQq")
    SQk = sb.tile([HD, N], F32, tag="SQk")
    NRMq = sb.tile([1, N], F32, tag="NRMq")
    NRMk = sb.tile([1, N], F32, tag="NRMk")
    LNq = sb.tile([1, N], F32, tag="LNq")
    LNk = sb.tile([1, N], F32, tag="LNk")
    rq = sb.tile([1, N], F32R, tag="rq")
    rk = sb.tile([1, N], F32R, tag="rk")
    if O_FP8:
        Vaug = sb.tile([128, 16, 2, 16], FP8, tag="Vaug")
    else:
        Vaug = sb.tile([128, NKB, HD + 1], F32R, tag="Vaug")

    # ---- projections (channel-major, stacked replicas) ----
    # raw q/k go into Qst/Kst; normalized in place later
    for c in range(NCHUNK):
        sl = slice(c * QC, (c + 1) * QC)
        pq = pS.tile([104, QC], F32, tag="S", name=f"pq{rep}_{c}")
        nc.tensor.matmul(pq[:], lhsT=WQKV[:, 0:104], rhs=X[:, sl], start=True, stop=True)
        nc.vector.tensor_copy(Qst[:, sl], pq[:])
        pk = pS.tile([104, QC], F32, tag="S", name=f"pk{rep}_{c}")
        nc.tensor.matmul(pk[:], lhsT=WQKV[:, 104:208], rhs=Y[:, sl], start=True, stop=True)
        nc.vector.tensor_copy(Kst[:, sl], pk[:])
        pv = pS.tile([HD, QC], F32, tag="O", name=f"pv{rep}_{c}")
        nc.tensor.matmul(pv[:], lhsT=WQKV[:, 208:216], rhs=Y[:, sl], start=True, stop=True)
        nc.vector.tensor_copy(Vc[:, sl], pv[:])

    # ---- norms: ssq -> 1/sqrt via ln/exp (same ACT table set as softmax) ----
    nc.vector.tensor_mul(SQq[:], Qst[0:HD, :], Qst[0:HD, :])
    nc.vector.tensor_mul(SQk[:], Kst[0:HD, :], Kst[0:HD, :])
    for c in range(NCHUNK):
        sl = slice(c * QC, (c + 1) * QC)
        nq = pF.tile([1, QC], F32, tag="F", name=f"nq{rep}_{c}")
        nc.tensor.matmul(nq[:], lhsT=ones8[:], rhs=SQq[:, sl], start=True, stop=True)
        nc.vector.tensor_copy(NRMq[:, sl], nq[:])
        nk = pB.tile([1, QC], F32, tag="B", name=f"nk{rep}_{c}")
        nc.tensor.matmul(nk[:], lhsT=ones8[:], rhs=SQk[:, sl], start=True, stop=True)
        nc.vector.tensor_copy(NRMk[:, sl], nk[:])
    nc.scalar.activation(LNq[:], NRMq[:], Ln, bias=0.0)
    nc.scalar.activation(LNk[:], NRMk[:], Ln, bias=0.0)
    with nc.allow_low_precision(reason="f32r for matmul rhs"):
        # rq = temp / |q| ; rk = 1 / |k|
        nc.scalar.activation(rq[:], LNq[:], Exp, bias=0.0, scale=-0.5)
        nc.scalar.activation(rk[:], LNk[:], Exp, bias=0.0, scale=-0.5)
        nc.vector.tensor_scalar_mul(rq[:], in0=rq[:], scalar1=BC[:, 208:209])

    # ---- normalize q/k in place (broadcast 1/norm via K=1 matmul) ----
    for c in range(NCHUNK):
        sl = slice(c * QC, (c + 1) * QC)
        bq = pS.tile([104, QC], F32, tag="S", name=f"bq{rep}_{c}")
        nc.tensor.matmul(bq[:], lhsT=BC[:, 0:104], rhs=rq[:, sl], start=True, stop=True)
        bk = pS.tile([104, QC], F32, tag="S", name=f"bk{rep}_{c}")
        nc.tensor.matmul(bk[:], lhsT=BC[:, 104:208], rhs=rk[:, sl], start=True, stop=True)
        with nc.allow_low_precision(reason="f32r for matmul inputs"):
            nc.vector.tensor_mul(Qst[:, sl], Qst[:, sl], bq[:])
            nc.vector.tensor_mul(Kst[:, sl], Kst[:, sl], bk[:])

    # ---- V to token-major (PE transposes), augmented with ones column ----
    from concourse.masks import make_identity

    ident = sb.tile([128, 128], F32, tag="ident")
    make_identity(nc, ident[:])
    vtr = pB.tile([128, 256], F32, tag="B", name=f"vtr{rep}")
    for kb in range(NKB):
        nc.tensor.transpose(
            vtr[:, kb * HD : (kb + 1) * HD],
            Vc[:, kb * 128 : (kb + 1) * 128],
            ident[:],
        )
    if O_FP8:
        # pairs (kb0,kb1) interleaved: Vaug[key, pair, i, 0:8] = V[c, ...]
        va = Vaug[:].bitcast(FP8)
        nc.vector.tensor_copy(
            bass.AP(
                tensor=va.tensor,
                offset=va.offset,
                ap=[[512, 128], [32, 16], [16, 2], [1, HD]],
            ),
            vtr[:].rearrange("p (a b) -> p a b", b=HD),
        )
        nc.vector.memset(Vaug[:, :, :, HD : HD + 1].bitcast(U8), 0)
        nc.vector.memset(Vaug[:, :, :, HD : HD + 1], 1.0)
    else:
        nc.vector.tensor_copy(
            Vaug[:, :, 0:HD], vtr[:].rearrange("p (a b) -> p a b", b=HD)
        )
        nc.vector.memset(Vaug[:, :, HD : HD + 1].bitcast(F32), 1.0)

    # ---- main loop ----
    for qc in range(NQC):
        qsl = slice(qc * QC, (qc + 1) * QC)
        O = pO.tile([128, QC], F32, tag="O", name=f"O{rep}_{qc}")
        for gr in range(16):  # granule = 2 key blocks
            S = pS.tile([128, 2 * QC], F32, tag="S", name=f"S{rep}_{qc}_{gr}")
            P = Pp.tile([128, 2 * QC], FP8 if O_FP8 else F32R, tag="P")
            for j in range(2):
                kb = gr * 2 + j
                b = 32 * (kb % 4)
                nc.tensor.matmul(
                    S[:, j * QC : (j + 1) * QC],
                    lhsT=Kst[b : b + HD, kb * 128 : (kb + 1) * 128],
                    rhs=Qst[b : b + HD, qsl],
                    start=True,
                    stop=True,
                    tile_position=(b, 0),
                )
            nc.scalar.activation(P[:], S[:], Exp, bias=0.0)
            if O_FP8:
                nc.tensor.matmul(
                    O[0 : HD + 1, :],
                    lhsT=Vaug[:, gr, :, 0 : HD + 1],
                    rhs=P[:].rearrange("p (a b) -> p a b", b=QC),
                    start=(gr == 0),
                    stop=(gr == 15),
                    perf_mode=mybir.MatmulPerfMode.DoubleRow,
                    skip_group_check=True,
                )
            else:
                for j in range(2):
                    kb = gr * 2 + j
                    nc.tensor.matmul(
                        O[0 : HD + 1, :],
                        lhsT=Vaug[:, kb, :],
                        rhs=P[:, j * QC : (j + 1) * QC],
                        start=(kb == 0),
                        stop=(kb == NKB - 1),
                        skip_group_check=True,
                    )

        # epilogue: project + merge denominator; divide via K=1 broadcast mm
        O_sb = epi.tile([HD + 1, QC], F32R, tag="O_sb")
        with nc.allow_low_precision(reason="f32r for matmul rhs"):
            nc.vector.tensor_copy(O_sb[:], O[0 : HD + 1, :])
        proj = pF.tile([DIM + 1, QC], F32, tag="F", name=f"proj{rep}_{qc}")
        nc.tensor.matmul(proj[:], lhsT=WOT[:], rhs=O_sb[:], start=True, stop=True)
        rden = epi.tile([1, QC], F32R, tag="rden")
        with nc.allow_low_precision(reason="f32r for matmul rhs"):
            nc.vector.reciprocal(rden[:], proj[DIM : DIM + 1, :])
        rdenb = pB.tile([DIM, QC], F32, tag="B", name=f"rdenb{rep}_{qc}")
        nc.tensor.matmul(
            rdenb[:], lhsT=BC[:, 209:210], rhs=rden[:], start=True, stop=True
        )
        rdenb_sb = epi.tile([DIM, QC], F32, tag="rdenb_sb")
        nc.scalar.activation(rdenb_sb[:], rdenb[:], Copy, bias=0.0)
        res = epi.tile([DIM, QC], F32, tag="res")
        nc.vector.tensor_mul(res[:], proj[0:DIM, :], rdenb_sb[:])
        (nc.sync if qc % 2 == 0 else nc.gpsimd).dma_start(out_d[:, qsl], res[:])


def build_program(reps: int = 1):
    _apply_walrus_compat()
    nc = bass.Bass("TRN2", target_bir_lowering=False, debug=False)
    x_d = nc.dram_tensor("x", [DIM, N], F32, kind="ExternalInput").ap()
    y_d = nc.dram_tensor("y", [DIM, N], F32, kind="ExternalInput").ap()
    wqkv_d = nc.dram_tensor("wqkv", [DIM, 216], F32, kind="ExternalInput").ap()
    wot_d = nc.dram_tensor("wot", [HD + 1, DIM + 1], F32, kind="ExternalInput").ap()
    bc_d = nc.dram_tensor("bc", [1, 211], F32, kind="ExternalInput").ap()
    outs = []
    with tile.TileContext(nc) as tc:
        import contextlib

        ctx = contextlib.ExitStack()
        with ctx:
            sb = ctx.enter_context(tc.tile_pool(name="sb", bufs=1))
            pS = ctx.enter_context(tc.tile_pool(name="pS", bufs=2, space="PSUM"))
            pO = ctx.enter_context(tc.tile_pool(name="pO", bufs=2, space="PSUM"))
            pF = ctx.enter_context(tc.tile_pool(name="pF", bufs=1, space="PSUM"))
            pB = ctx.enter_context(tc.tile_pool(name="pB", bufs=1, space="PSUM"))
            Pp = ctx.enter_context(tc.tile_pool(name="Pp", bufs=3))
            epi = ctx.enter_context(tc.tile_pool(name="epi", bufs=2))
            pools = (sb, pS, pO, pF, pB, Pp, epi)
            for rep in range(reps):
                out_d = nc.dram_tensor(
                    f"out{rep}", [DIM, N], F32, kind="ExternalOutput"
                ).ap()
                outs.append(f"out{rep}")
                _emit_rep(nc, tc, rep, x_d, y_d, wqkv_d, wot_d, bc_d, out_d, pools)
    return nc, outs


def make_in_maps(x, y, w_q, w_kv, w_out, temperature):
    x = np.ascontiguousarray(np.asarray(x, dtype=np.float32))
    y = np.ascontiguousarray(np.asarray(y, dtype=np.float32))
    w_q = np.asarray(w_q, dtype=np.float32)
    w_kv = np.asarray(w_kv, dtype=np.float32)
    w_out = np.asarray(w_out, dtype=np.float32)
    temperature = np.asarray(temperature, dtype=np.float32).reshape(NUM_HEADS)
    assert x.shape == (1, DIM, 64, 64) and y.shape == (1, DIM, 64, 64)
    X = x.reshape(DIM, N)
    Y = y.reshape(DIM, N)
    # broadcast/selection constants
    bcq = np.zeros((1, 104), dtype=np.float32)
    bck = np.zeros((1, 104), dtype=np.float32)
    for g in range(4):
        bcq[0, 32 * g : 32 * g + HD] = 1.0
        bck[0, 32 * g : 32 * g + HD] = 1.0
    in_maps = []
    for h in range(NUM_HEADS):
        sl = slice(h * HD, (h + 1) * HD)
        wq4 = np.zeros((DIM, 104), dtype=np.float32)
        wk4 = np.zeros((DIM, 104), dtype=np.float32)
        for g in range(4):
            wq4[:, 32 * g : 32 * g + HD] = w_q[sl].T
            wk4[:, 32 * g : 32 * g + HD] = w_kv[sl].T
        wv = w_kv[DIM + h * HD : DIM + (h + 1) * HD].T  # [64, 8]
        wqkv = np.concatenate([wq4, wk4, wv], axis=1)  # [64, 216]
        wot = np.zeros((HD + 1, DIM + 1), dtype=np.float32)
        wot[0:HD, 0:DIM] = w_out[:, sl].T
        wot[HD, DIM] = 1.0
        bc = np.zeros((1, 211), dtype=np.float32)
        bc[0, 0:104] = bcq
        bc[0, 104:208] = bck
        bc[0, 208] = temperature[h]
        bc[0, 209] = 1.0  # lhsT for denominator broadcast (K=1 -> 64 rows)
        in_maps.append(
            {
                "x": X,
                "y": Y,
                "wqkv": np.ascontiguousarray(wqkv),
                "wot": wot,
                "bc": bc,
            }
        )
    return in_maps


def kernel(x, y, w_q, w_kv, w_out, temperature):
    from concourse.bass_utils import run_bass_kernel_spmd

    nc, out_names = build_program(reps=1)
    in_maps = make_in_maps(x, y, w_q, w_kv, w_out, temperature)
    res = run_bass_kernel_spmd(nc, in_maps, list(range(NUM_HEADS)))
    total = np.zeros((DIM, N), dtype=np.float32)
    for h in range(NUM_HEADS):
        total += res.results[h][out_names[0]]
    return total.reshape(1, DIM, 64, 64)


# revision 5
# speedup vs baseline: 1.1136x; 1.1136x over previous
"""Trainium2 Bass kernel v2 for nn_AttentionSpatial (spatial cosine attention).

Per head h (8 heads, head h -> core h):
  q = W_q X, k/v = W_kv Y                (channel-major, [8, 4096])
  q' = q/|q| * temp, k' = k/|k|          (norms via ones-matmul + ACT ln/exp)
  S[key, qcol] = k'.T q'                 (row-packed: 4 key-blocks concurrent
                                          in PE row groups 0/32/64/96)
  P = exp(S)                             (bounded logits => no max pass)
  O = [V | 1].T P                        (fp8 DoubleRow pairs two key blocks
                                          per matmul, or f32r classic)
  out = (W_out O) / den                  (den broadcast via K=1 matmul)

Layouts:
  Qst/Kst [104, 4096] f32r: channels replicated at partition 0/32/64/96
    (produced directly by stacked projection weights wq4/wk4 [64, 104]).
  Vaug fp8 [128, 16, 2, 16]: token-major value pairs (+ones col 8) for
    DoubleRow; or f32r [128, 32, 9] for classic mode.
"""

import numpy as np

import concourse.bass as bass
import concourse.tile as tile
from concourse import mybir

NUM_HEADS = 8
DIM = 64
HD = 8
N = 4096
NCHUNK = 8  # 512-column chunks
QC = 512
NQC = N // QC
NKB = 32  # 128-key blocks
F32 = mybir.dt.float32
F32R = mybir.dt.float32r
FP8 = mybir.dt.float8e4
U8 = mybir.dt.uint8

import os

O_FP8 = os.environ.get("KERN_O_FP8", "1") == "1"

_patched = False


def _apply_walrus_compat():
    """This container's walrus build rejects Drain instructions that carry
    sync waits ("Too many sync wait commands").  Replace multi-engine
    barriers with the sem-only variant and re-emit the TileContext tail
    drain's waits as standalone EventSemaphore instructions."""
    global _patched
    if _patched:
        return
    _patched = True
    from concourse.vector_clock import ScopedClock

    def meb(self, engines):
        for e in engines:
            self.engines[e].drain()
        for inst in self._sem_only_all_engine_barrier_insts("meb"):
            self.engines[inst.engine].add_instruction(inst)

    bass.Bass.multi_engine_barrier = meb

    def _drain_and_barrier(self, tick_clock, wait_clock):
        nc = self.nc
        carrier = nc.sync.nop()
        wait_clock.add_sem_waits(
            carrier.ins, ScopedClock({None: tick_clock.global_clock})
        )
        si = carrier.ins.sync_info
        waits = list(si.on_wait) if si and si.on_wait else []
        if si is not None:
            si.on_wait = []
        sems = list(self.sems.allocated().values())
        placeholder = sems[0] if sems else nc.alloc_semaphore("tailw")
        for w in waits:
            assert w.wait_mode in ("sem-ge-imm", "sem-ge"), w.wait_mode
            ev = nc.sync.wait_ge(placeholder, 0)
            ev.ins.sync_info.on_wait = [w]
        nc.sync.drain()
        nc.all_engine_barrier()
        popped = nc._tile_sem_poison_stack.pop()
        assert popped is self._sem_poison
        nc.clear_and_free_semaphores(list(self.sems.allocated().values()))
        nc.all_engine_barrier()

    tile.TileContext._drain_and_barrier = _drain_and_barrier

    orig_commit = tile.TileContext._commit_instruction

    def _commit_instruction(self, inst, lazy_reg_writes=True):
        si = inst.sync_info
        if si is not None and si.on_wait:
            is_drain = type(inst).__name__ == "InstDrain"
            waits = list(si.on_wait)
            n_ge = sum(
                1 for w in waits if w.wait_mode in ("sem-ge-imm", "sem-ge")
            )
            assert n_ge == len(waits) or not is_drain, f"eq-wait on drain {inst}"
            keep = 0 if is_drain else 1
            if len(waits) > keep and inst.engine != mybir.EngineType.Unassigned:
                kept, split = waits[:keep], waits[keep:]
                si.on_wait = kept
                sems = list(self.sems.allocated().values())
                placeholder = sems[0] if sems else self.nc.alloc_semaphore("splitw")
                eng = self.nc.engines[inst.engine]
                for w in split:
                    assert w.wait_mode in ("sem-ge-imm", "sem-ge"), w.wait_mode
                    ev = eng.wait_ge(placeholder, 0)
                    ev.ins.sync_info.on_wait = [w]
        return orig_commit(self, inst, lazy_reg_writes)

    tile.TileContext._commit_instruction = _commit_instruction


def _emit_rep(nc, tc, rep, x_d, y_d, wqkv_d, wot_d, bc_d, out_d, pools):
    Exp = mybir.ActivationFunctionType.Exp
    Ln = mybir.ActivationFunctionType.Ln
    Copy = mybir.ActivationFunctionType.Copy
    sb, pS, pO, pF, Pp, epi = pools

    # ---- load inputs ----
    X = sb.tile([DIM, N], F32R, tag="X")
    Y = sb.tile([DIM, N], F32R, tag="Y")
    WQKV = sb.tile([DIM, 216], F32R, tag="WQKV")  # wq4 | wk4 | wv
    WOT = sb.tile([HD + 1, DIM + 1], F32R, tag="WOT")
    BC = sb.tile([1, 211], F32R, tag="BC")  # bcq(104) | bck(104) | temp | ones(2)
    nc.sync.dma_start(X[0:32, :].bitcast(F32), x_d[0:32, :])
    nc.gpsimd.dma_start(X[32:64, :].bitcast(F32), x_d[32:64, :])
    nc.sync.dma_start(Y[0:32, :].bitcast(F32), y_d[0:32, :])
    nc.gpsimd.dma_start(Y[32:64, :].bitcast(F32), y_d[32:64, :])
    nc.sync.dma_start(WQKV[:].bitcast(F32), wqkv_d[:])
    nc.sync.dma_start(WOT[:].bitcast(F32), wot_d[:])
    nc.sync.dma_start(BC[:].bitcast(F32), bc_d[:])
    ones8 = sb.tile([HD, 1], F32R, tag="ones8")
    nc.vector.memset(ones8[:].bitcast(F32), 1.0)

    # persistent SBUF state
    Qst = sb.tile([104, N], F32R, tag="Qst")
    Kst = sb.tile([104, N], F32R, tag="Kst")
    Vc = sb.tile([HD, N], F32R, tag="Vc")
    SQq = sb.tile([HD, N], F32, tag="SQq")
    SQk = sb.tile([HD, N], F32, tag="SQk")
    NRMq = sb.tile([1, N], F32, tag="NRMq")
    NRMk = sb.tile([1, N], F32, tag="NRMk")
    LNq = sb.tile([1, N], F32, tag="LNq")
    LNk = sb.tile([1, N], F32, tag="LNk")
    rq = sb.tile([1, N], F32R, tag="rq")
    rk = sb.tile([1, N], F32R, tag="rk")
    if O_FP8:
        Vaug = sb.tile([128, 16, 2, 16], FP8, tag="Vaug")
    else:
        Vaug = sb.tile([128, NKB, HD + 1], F32R, tag="Vaug")

    # ---- projections (channel-major, stacked replicas) ----
    # raw q/k go into Qst/Kst; normalized in place later
    for c in range(NCHUNK):
        sl = slice(c * QC, (c + 1) * QC)
        pq = pS.tile([104, QC], F32, tag="S", name=f"pq{rep}_{c}")
        nc.tensor.matmul(pq[:], lhsT=WQKV[:, 0:104], rhs=X[:, sl], start=True, stop=True)
        nc.vector.tensor_copy(Qst[:, sl], pq[:])
        pk = pS.tile([104, QC], F32, tag="S", name=f"pk{rep}_{c}")
        nc.tensor.matmul(pk[:], lhsT=WQKV[:, 104:208], rhs=Y[:, sl], start=True, stop=True)
        nc.vector.tensor_copy(Kst[:, sl], pk[:])
        pv = pS.tile([HD, QC], F32, tag="O", name=f"pv{rep}_{c}")
        nc.tensor.matmul(pv[:], lhsT=WQKV[:, 208:216], rhs=Y[:, sl], start=True, stop=True)
        nc.vector.tensor_copy(Vc[:, sl], pv[:])

    # ---- norms: ssq -> 1/sqrt via ln/exp (same ACT table set as softmax) ----
    nc.vector.tensor_mul(SQq[:], Qst[0:HD, :], Qst[0:HD, :])
    nc.vector.tensor_mul(SQk[:], Kst[0:HD, :], Kst[0:HD, :])
    for c in range(NCHUNK):
        sl = slice(c * QC, (c + 1) * QC)
        nq = pF.tile([1, QC], F32, tag="F", name=f"nq{rep}_{c}")
        nc.tensor.matmul(nq[:], lhsT=ones8[:], rhs=SQq[:, sl], start=True, stop=True)
        nc.vector.tensor_copy(NRMq[:, sl], nq[:])
        nk = pB.tile([1, QC], F32, tag="B", name=f"nk{rep}_{c}")
        nc.tensor.matmul(nk[:], lhsT=ones8[:], rhs=SQk[:, sl], start=True, stop=True)
        nc.vector.tensor_copy(NRMk[:, sl], nk[:])
    nc.scalar.activation(LNq[:], NRMq[:], Ln, bias=0.0)
    nc.scalar.activation(LNk[:], NRMk[:], Ln, bias=0.0)
    with nc.allow_low_precision(reason="f32r for matmul rhs"):
        # rq = temp / |q| ; rk = 1 / |k|
        nc.scalar.activation(rq[:], LNq[:], Exp, bias=0.0, scale=-0.5)
        nc.scalar.activation(rk[:], LNk[:], Exp, bias=0.0, scale=-0.5)
        nc.vector.tensor_scalar_mul(rq[:], in0=rq[:], scalar1=BC[:, 208:209])

    # ---- normalize q/k in place (broadcast 1/norm via K=1 matmul) ----
    for c in range(NCHUNK):
        sl = slice(c * QC, (c + 1) * QC)
        bq = pS.tile([104, QC], F32, tag="S", name=f"bq{rep}_{c}")
        nc.tensor.matmul(bq[:], lhsT=BC[:, 0:104], rhs=rq[:, sl], start=True, stop=True)
        bk = pS.tile([104, QC], F32, tag="S", name=f"bk{rep}_{c}")
        nc.tensor.matmul(bk[:], lhsT=BC[:, 104:208], rhs=rk[:, sl], start=True, stop=True)
        with nc.allow_low_precision(reason="f32r for matmul inputs"):
            nc.vector.tensor_mul(Qst[:, sl], Qst[:, sl], bq[:])
            nc.vector.tensor_mul(Kst[:, sl], Kst[:, sl], bk[:])

    # ---- V to token-major (PE transposes), augmented with ones column ----
    from concourse.masks import make_identity

    ident = sb.tile([128, 128], F32, tag="ident")
    make_identity(nc, ident[:])
    vtr = pB.tile([128, 256], F32, tag="B", name=f"vtr{rep}")
    for kb in range(NKB):
        nc.tensor.transpose(
            vtr[:, kb * HD : (kb + 1) * HD],
            Vc[:, kb * 128 : (kb + 1) * 128],
            ident[:],
        )
    if O_FP8:
        # pairs (kb0,kb1) interleaved: Vaug[key, pair, i, 0:8] = V[c, ...]
        va = Vaug[:].bitcast(FP8)
        nc.vector.tensor_copy(
            bass.AP(
                tensor=va.tensor,
                offset=va.offset,
                ap=[[512, 128], [32, 16], [16, 2], [1, HD]],
            ),
            vtr[:].rearrange("p (a b) -> p a b", b=HD),
        )
        nc.vector.memset(Vaug[:, :, :, HD : HD + 1].bitcast(U8), 0)
        nc.vector.memset(Vaug[:, :, :, HD : HD + 1], 1.0)
    else:
        nc.vector.tensor_copy(
            Vaug[:, :, 0:HD], vtr[:].rearrange("p (a b) -> p a b", b=HD)
        )
        nc.vector.memset(Vaug[:, :, HD : HD + 1].bitcast(F32), 1.0)

    # ---- main loop ----
    for qc in range(NQC):
        qsl = slice(qc * QC, (qc + 1) * QC)
        O = pO.tile([128, QC], F32, tag="O", name=f"O{rep}_{qc}")
        for gr in range(16):  # granule = 2 key blocks
            S = pS.tile([128, 2 * QC], F32, tag="S", name=f"S{rep}_{qc}_{gr}")
            P = Pp.tile([128, 2 * QC], FP8 if O_FP8 else F32R, tag="P")
            for j in range(2):
                kb = gr * 2 + j
                b = 32 * (kb % 4)
                nc.tensor.matmul(
                    S[:, j * QC : (j + 1) * QC],
                    lhsT=Kst[b : b + HD, kb * 128 : (kb + 1) * 128],
                    rhs=Qst[b : b + HD, qsl],
                    start=True,
                    stop=True,
                    tile_position=(b, 0),
                )
            nc.scalar.activation(P[:], S[:], Exp, bias=0.0)
            if O_FP8:
                nc.tensor.matmul(
                    O[0 : HD + 1, :],
                    lhsT=Vaug[:, gr, :, 0 : HD + 1],
                    rhs=P[:].rearrange("p (a b) -> p a b", b=QC),
                    start=(gr == 0),
                    stop=(gr == 15),
                    perf_mode=mybir.MatmulPerfMode.DoubleRow,
                    skip_group_check=True,
                )
            else:
                for j in range(2):
                    kb = gr * 2 + j
                    nc.tensor.matmul(
                        O[0 : HD + 1, :],
                        lhsT=Vaug[:, kb, :],
                        rhs=P[:, j * QC : (j + 1) * QC],
                        start=(kb == 0),
                        stop=(kb == NKB - 1),
                        skip_group_check=True,
                    )

        # epilogue: project + merge denominator; divide via K=1 broadcast mm
        O_sb = epi.tile([HD + 1, QC], F32R, tag="O_sb")
        with nc.allow_low_precision(reason="f32r for matmul rhs"):
            nc.vector.tensor_copy(O_sb[:], O[0 : HD + 1, :])
        proj = pF.tile([DIM + 1, QC], F32, tag="F", name=f"proj{rep}_{qc}")
        nc.tensor.matmul(proj[:], lhsT=WOT[:], rhs=O_sb[:], start=True, stop=True)
        rden = epi.tile([1, QC], F32R, tag="rden")
        with nc.allow_low_precision(reason="f32r for matmul rhs"):
            nc.vector.reciprocal(rden[:], proj[DIM : DIM + 1, :])
        rdenb = pO.tile([DIM, QC], F32, tag="O", name=f"rdenb{rep}_{qc}")
        nc.tensor.matmul(
            rdenb[:], lhsT=BC[:, 209:210], rhs=rden[:], start=True, stop=True
        )
        rdenb_sb = epi.tile([DIM, QC], F32, tag="rdenb_sb")
        nc.scalar.activation(rdenb_sb[:], rdenb[:], Copy, bias=0.0)
        res = epi.tile([DIM, QC], F32, tag="res")
        nc.vector.tensor_mul(res[:], proj[0:DIM, :], rdenb_sb[:])
        (nc.sync if qc % 2 == 0 else nc.gpsimd).dma_start(out_d[:, qsl], res[:])


def build_program(reps: int = 1):
    _apply_walrus_compat()
    nc = bass.Bass("TRN2", target_bir_lowering=False, debug=False)
    x_d = nc.dram_tensor("x", [DIM, N], F32, kind="ExternalInput").ap()
    y_d = nc.dram_tensor("y", [DIM, N], F32, kind="ExternalInput").ap()
    wqkv_d = nc.dram_tensor("wqkv", [DIM, 216], F32, kind="ExternalInput").ap()
    wot_d = nc.dram_tensor("wot", [HD + 1, DIM + 1], F32, kind="ExternalInput").ap()
    bc_d = nc.dram_tensor("bc", [1, 211], F32, kind="ExternalInput").ap()
    outs = []
    with tile.TileContext(nc) as tc:
        import contextlib

        ctx = contextlib.ExitStack()
        with ctx:
            sb = ctx.enter_context(tc.tile_pool(name="sb", bufs=1))
            pS = ctx.enter_context(tc.tile_pool(name="pS", bufs=3, space="PSUM"))
            pO = ctx.enter_context(tc.tile_pool(name="pO", bufs=2, space="PSUM"))
            pF = ctx.enter_context(tc.tile_pool(name="pF", bufs=1, space="PSUM"))
            Pp = ctx.enter_context(tc.tile_pool(name="Pp", bufs=3))
            epi = ctx.enter_context(tc.tile_pool(name="epi", bufs=2))
            pools = (sb, pS, pO, pF, Pp, epi)
            for rep in range(reps):
                out_d = nc.dram_tensor(
                    f"out{rep}", [DIM, N], F32, kind="ExternalOutput"
                ).ap()
                outs.append(f"out{rep}")
                _emit_rep(nc, tc, rep, x_d, y_d, wqkv_d, wot_d, bc_d, out_d, pools)
    return nc, outs


def make_in_maps(x, y, w_q, w_kv, w_out, temperature):
    x = np.ascontiguousarray(np.asarray(x, dtype=np.float32))
    y = np.ascontiguousarray(np.asarray(y, dtype=np.float32))
    w_q = np.asarray(w_q, dtype=np.float32)
    w_kv = np.asarray(w_kv, dtype=np.float32)
    w_out = np.asarray(w_out, dtype=np.float32)
    temperature = np.asarray(temperature, dtype=np.float32).reshape(NUM_HEADS)
    assert x.shape == (1, DIM, 64, 64) and y.shape == (1, DIM, 64, 64)
    X = x.reshape(DIM, N)
    Y = y.reshape(DIM, N)
    # broadcast/selection constants
    bcq = np.zeros((1, 104), dtype=np.float32)
    bck = np.zeros((1, 104), dtype=np.float32)
    for g in range(4):
        bcq[0, 32 * g : 32 * g + HD] = 1.0
        bck[0, 32 * g : 32 * g + HD] = 1.0
    in_maps = []
    for h in range(NUM_HEADS):
        sl = slice(h * HD, (h + 1) * HD)
        wq4 = np.zeros((DIM, 104), dtype=np.float32)
        wk4 = np.zeros((DIM, 104), dtype=np.float32)
        for g in range(4):
            wq4[:, 32 * g : 32 * g + HD] = w_q[sl].T
            wk4[:, 32 * g : 32 * g + HD] = w_kv[sl].T
        wv = w_kv[DIM + h * HD : DIM + (h + 1) * HD].T  # [64, 8]
        wqkv = np.concatenate([wq4, wk4, wv], axis=1)  # [64, 216]
        wot = np.zeros((HD + 1, DIM + 1), dtype=np.float32)
        wot[0:HD, 0:DIM] = w_out[:, sl].T
        wot[HD, DIM] = 1.0
        bc = np.zeros((1, 211), dtype=np.float32)
        bc[0, 0:104] = bcq
        bc[0, 104:208] = bck
        bc[0, 208] = temperature[h]
        bc[0, 209] = 1.0  # lhsT for denominator broadcast (K=1 -> 64 rows)
        in_maps.append(
            {
                "x": X,
                "y": Y,
                "wqkv": np.ascontiguousarray(wqkv),
                "wot": wot,
                "bc": bc,
            }
        )
    return in_maps


def kernel(x, y, w_q, w_kv, w_out, temperature):
    from concourse.bass_utils import run_bass_kernel_spmd

    nc, out_names = build_program(reps=1)
    in_maps = make_in_maps(x, y, w_q, w_kv, w_out, temperature)
    res = run_bass_kernel_spmd(nc, in_maps, list(range(NUM_HEADS)))
    total = np.zeros((DIM, N), dtype=np.float32)
    for h in range(NUM_HEADS):
        total += res.results[h][out_names[0]]
    return total.reshape(1, DIM, 64, 64)
